# revision 26
# baseline (speedup 1.0000x reference)
"""Trainium2 Bass kernel for the MultiLatentAttention (dense transformer) block.

Computes, for x:(4,2048,2048), mask:(4,1,2048,2048):
    q/k/v = x @ W{q,k,v} + b  (per-head, head_dim=128, 16 heads)
    q,k <- interleaved RoPE
    attn = softmax(q k^T / sqrt(2048)) * mask
    out  = (attn @ v) @ Wo + bo

Sharding: 8 cores = 4 batches x 2 head-groups (8 heads each). Each core
computes its batch's q/k/v for its 8 heads, attention, and a partial
o-projection (row-parallel over Wo). Host sums the two partials per batch
and adds bo. No device collectives.

Numerics / layout:
 - q/k projections run in fp8e4 with perf_mode=DoubleRow (256-deep
   contraction per pass, 2x the fp16 matmul rate on HW). Wq/Wk are
   scaled x32 on host so their values sit in e4m3's normal range; the
   x1024 scores scale is folded into the exp() scale. End-to-end
   max-rel-err of this scheme is 1.28e-2 (gate: 2e-2); everything else
   runs in fp16 which alone would land at 5.4e-4.
 - v projection / scores / attn@v / o-projection all use fp16 operands
   (fp32 PSUM accumulate). The 16-bit mix keeps LDWEIGHTS hidden under
   the matmul stream and (vs the fp32r baseline) keeps the PE's power
   throttle at the full 2.4 GHz clock instead of 13/16.
 - RoPE interleaved pairs are de-interleaved by permuting W{q,k} columns
   per head (even dims -> partitions 0..63, odd -> 64..127); q.k is
   invariant under a shared head-dim permutation and v/Wo are left
   unpermuted. Rotation sign is folded into the sin table.
 - softmax has no max-subtraction (scores are O(1)); the denominator is
   accumulated across k-blocks on DVE in fp16 quad tiles ([128,4,512]),
   then 4 accumulating ones-stationary matmuls per (qc, head) produce
   the partition-broadcast sum (vs a per-k-block ones matmul).

Scheduling (phases B -> A -> C; engine balance is the whole game):
 - B: q/k DoubleRow projections, head-pair outer / query-chunk inner so
   the first pass over the fp8 activations is DMA-paced across four
   head-projections instead of one. Weights for the next pair prefetch
   during the current one.
 - A: v projection (x fp16 stationary). Its first activation chunk and
   both Wv column groups preload during B into carved-out pools, so only
   the later chunks ride the space vacated by B's pools.
 - C: attention, qc (512 query cols) outer, heads inner. exp runs on
   ACT over [128,1024] PSUM pairs; mask-multiply and the denominator
   accumulation on DVE; attn@v and scores on the PE. The o-projection
   of the PREVIOUS qc is split into 16 single-bank subgroups and
   interleaved two-per-head-unit as dependency-free PE filler for the
   exp-wait stalls; the final qc's o-proj runs at the end.
 - Bulk loads ride the second HWDGE ring (ACT engine); latency-critical
   small loads and output spills use the sync ring. All working tensors
   (q/k 8MB fp16, v 4MB, per-qc mask, Wo) stay resident in SBUF; only
   fp16 output partials are spilled.
"""

import numpy as np

B, S, H, NH = 4, 2048, 2048, 16
D = 128            # head dim
G = 2              # head groups (tensor-parallel)
HL = NH // G       # heads per core = 8
P = 128
KO = H // P        # 16 contraction blocks
KOP = KO // 2      # 8 DoubleRow contraction pairs
SB = S // P        # 16 sequence blocks
NQ = S // 512      # 4 query-column chunks
ROPE_BASE = 10000.0
WSCALE = 32.0      # host scale on Wq/Wk (and bq/bk) for fp8 range
SCALE = 1.0 / np.sqrt(np.float32(H))
EXP_SCALE = float(SCALE / (WSCALE * WSCALE))

_CACHE = {}


def _build_program(with_bv):
    import concourse.mybir as mybir
    import concourse.tile as tile
    from concourse import bacc

    f32 = mybir.dt.float32
    f16 = mybir.dt.float16
    f8 = mybir.dt.float8e4
    AF = mybir.ActivationFunctionType
    DR = mybir.MatmulPerfMode.DoubleRow
    ADD = mybir.AluOpType.add
    MULT = mybir.AluOpType.mult

    nc = bacc.Bacc("TRN2", num_devices=8, debug=False, num_swdge_queues=4)

    xtA = nc.dram_tensor("xtA", [P, KOP * 2 * S], f8, kind="ExternalInput")
    xtB = nc.dram_tensor("xtB", [P, KO * S], f16, kind="ExternalInput")
    maskQ = nc.dram_tensor("maskQ", [P, NQ * KO * 512], f16,
                           kind="ExternalInput")
    wq8 = nc.dram_tensor("wq8", [HL, P, KOP * 2 * D], f8, kind="ExternalInput")
    wk8 = nc.dram_tensor("wk8", [HL, P, KOP * 2 * D], f8, kind="ExternalInput")
    bqs = nc.dram_tensor("bqs", [P, HL], f32, kind="ExternalInput")
    bks = nc.dram_tensor("bks", [P, HL], f32, kind="ExternalInput")
    wv = nc.dram_tensor("wv", [P, 2 * KO * 512], f16, kind="ExternalInput")
    bv = nc.dram_tensor("bv", [P, HL * D], f16, kind="ExternalInput")
    wo = nc.dram_tensor("wo", [P, HL * H], f16, kind="ExternalInput")
    cosP = nc.dram_tensor("cosP", [P, S], f16, kind="ExternalInput")
    sinP = nc.dram_tensor("sinP", [P, S], f16, kind="ExternalInput")
    ones_d = nc.dram_tensor("ones", [P, P], f16, kind="ExternalInput")

    out = nc.dram_tensor("out", [S, H], f16, kind="ExternalOutput")

    xtA_r = xtA.rearrange("p (q kp i c) -> p q kp i c", q=NQ, kp=KOP, i=2)
    xtB_r = xtB.rearrange("p (q ko c) -> p q ko c", q=NQ, ko=KO)
    wv_r = wv.rearrange("p (g ko c) -> p g ko c", g=2, ko=KO)
    maskQ_r = maskQ.rearrange("p (q ko c) -> p q ko c", q=NQ, ko=KO)
    out_r = out.rearrange("(mo p) n -> mo p n", p=P)

    with tile.TileContext(nc) as tc:
        with (
            tc.tile_pool(name="vres", bufs=1) as vres_pool,
            tc.tile_pool(name="qkres", bufs=1) as qkres_pool,
            tc.tile_pool(name="cs", bufs=1) as cs_pool,
        ):
            v_sb = vres_pool.tile([P, SB, HL * D], f16, name="v_sb")
            qt_t = [
                qkres_pool.tile([P, S], f16, name=f"qt{h}", tag=f"qt{h}")
                for h in range(HL)
            ]
            kt_t = [
                qkres_pool.tile([P, S], f16, name=f"kt{h}", tag=f"kt{h}")
                for h in range(HL)
            ]
            ones_sb = cs_pool.tile([P, P], f16, name="ones_sb")

            # ---------------- phase B: q/k projections (fp8 DoubleRow) ----
            with (
                tc.tile_pool(name="wvp", bufs=2) as wv_pool,
                tc.tile_pool(name="xtb0", bufs=1) as xtb0_pool,
            ):
              with (
                tc.tile_pool(name="xta", bufs=1) as xta_pool,
                tc.tile_pool(name="w8", bufs=6) as w8_pool,
                tc.tile_pool(name="qps", bufs=6, space="PSUM") as qps_pool,
                tc.tile_pool(name="rp", bufs=2) as rp_pool,
              ):
                xta = xta_pool.tile([P, NQ, KOP, 2, 512], f8, name="xta")
                xtb_c0 = xtb0_pool.tile([P, KO, 512], f16, name="xtb_c0")
                cos_sb = xta_pool.tile([P, S], f16, name="cos_sb")
                sin_sb = xta_pool.tile([P, S], f16, name="sin_sb")
                bq_sb = xta_pool.tile([P, HL], f32, name="bq_sb")
                bk_sb = xta_pool.tile([P, HL], f32, name="bk_sb")
                def load_w(w_in, h):
                    wsb = w8_pool.tile([P, KOP, 2, D], f8, name="wsb",
                                       tag="w8")
                    nc.sync.dma_start(
                        wsb[:], w_in[h].rearrange("p (kp i d) -> p kp i d",
                                                  i=2, d=D)
                    )
                    return wsb

                # hp0's four weight tensors + the first xta column group go
                # on the latency-critical sync ring; bulk on the act ring
                wtiles = [load_w(wq8, 0)]
                nc.sync.dma_start(xta[:, 0, 0:2], xtA_r[:, 0, 0:2])
                nc.sync.dma_start(bq_sb[:], bqs[:, :])
                for kp2 in range(1, 4):
                    nc.sync.dma_start(xta[:, 0, 2 * kp2:2 * kp2 + 2],
                                      xtA_r[:, 0, 2 * kp2:2 * kp2 + 2])
                wtiles += [load_w(wk8, 0), load_w(wq8, 1), load_w(wk8, 1)]
                nc.sync.dma_start(bk_sb[:], bks[:, :])
                nc.scalar.dma_start(cos_sb[:], cosP[:, :])
                nc.scalar.dma_start(sin_sb[:], sinP[:, :])
                nc.scalar.dma_start(ones_sb[:], ones_d[:, :])
                for qc in range(1, NQ):
                    nc.scalar.dma_start(xta[:, qc], xtA_r[:, qc])
                wvt0 = wv_pool.tile([P, KO, 512], f16, name="wvt", tag="wv")
                wvt1 = wv_pool.tile([P, KO, 512], f16, name="wvt", tag="wv")

                for hp in range(HL // 2):
                    cur_w = wtiles
                    if hp + 1 < HL // 2:
                        wtiles = [load_w(w, 2 * hp + 2 + j)
                                  for j in (0, 1) for w in (wq8, wk8)]
                    if hp == 1:
                        nc.scalar.dma_start(wvt0[:], wv_r[:, 0])
                        nc.scalar.dma_start(wvt1[:], wv_r[:, 1])
                        nc.scalar.dma_start(xtb_c0[:], xtB_r[:, 0])
                    for qc in range(NQ):
                        for ti, (w_in, b_sb, dst) in enumerate((
                            (wq8, bq_sb, qt_t), (wk8, bk_sb, kt_t),
                            (wq8, bq_sb, qt_t), (wk8, bk_sb, kt_t),
                        )):
                            h = 2 * hp + ti // 2
                            wsb = cur_w[2 * (ti // 2) + (ti % 2)]
                            sl = slice(qc * 512, (qc + 1) * 512)
                            ps = qps_pool.tile([P, 512], f32, name="qkps",
                                               tag="qkps")
                            for kbp in range(KOP):
                                nc.tensor.matmul(
                                    ps[:],
                                    lhsT=wsb[:, kbp],
                                    rhs=xta[:, qc, kbp],
                                    start=(kbp == 0),
                                    stop=(kbp == KOP - 1),
                                    perf_mode=DR,
                                )
                            # rope drain: qt = qb*cos + swap64(qb)*sinP
                            qb = rp_pool.tile([P, 512], f16, name="qb", tag="qb")
                            nc.scalar.activation(
                                qb[:], ps[:], AF.Identity, bias=b_sb[:, h:h + 1]
                            )
                            qsw = rp_pool.tile([P, 512], f16, name="qsw",
                                               tag="qsw")
                            nc.scalar.copy(qsw[0:64], qb[64:128])
                            nc.vector.tensor_copy(qsw[64:128], qb[0:64])
                            t1 = rp_pool.tile([P, 512], f16, name="t1", tag="t1")
                            nc.vector.tensor_tensor(
                                t1[:], qb[:], cos_sb[:, sl], MULT
                            )
                            t2 = rp_pool.tile([P, 512], f16, name="t2", tag="t2")
                            nc.vector.tensor_tensor(
                                t2[:], qsw[:], sin_sb[:, sl], MULT
                            )
                            nc.vector.tensor_tensor(
                                dst[h][:, sl], t1[:], t2[:], ADD
                            )

              # -------------- phase A: v projection (fp16) --------------
              with (
                tc.tile_pool(name="xtb", bufs=1) as xtb_pool,
                tc.tile_pool(name="vps", bufs=4, space="PSUM") as vps_pool,
              ):
                xtb = xtb_pool.tile([P, NQ - 1, KO, 512], f16, name="xtb")
                if with_bv:
                    bv_sb = wv_pool.tile([P, HL * D], f16, name="bv_sb",
                                         tag="bv")
                    nc.sync.dma_start(bv_sb[:], bv[:, :])
                for c in range(1, NQ):
                    nc.sync.dma_start(xtb[:, c - 1], xtB_r[:, c])
                for cg in range(4):
                  for g2 in range(2):
                    wvt = wvt0 if g2 == 0 else wvt1
                    for sb in range(4 * cg, 4 * cg + 4):
                        ps = vps_pool.tile([P, 512], f32, name="vps",
                                           tag="vps")
                        for kb in range(KO):
                            nc.tensor.matmul(
                                ps[:],
                                lhsT=(xtb_c0 if sb < 4 else
                                      xtb[:, sb // 4 - 1])[
                                          :, kb, (sb % 4) * P:(sb % 4 + 1) * P],
                                rhs=wvt[:, kb],
                                start=(kb == 0),
                                stop=(kb == KO - 1),
                            )
                        dstv = v_sb[:, sb, g2 * 512:(g2 + 1) * 512]
                        if with_bv:
                            nc.vector.tensor_tensor(
                                dstv, ps[:],
                                bv_sb[:, g2 * 512:(g2 + 1) * 512], ADD
                            )
                        else:
                            nc.vector.tensor_copy(dstv, ps[:])

            # ---------------- phase C: attention + o-proj ----------------
            with (
                tc.tile_pool(name="wores", bufs=1) as wo_pool,
                tc.tile_pool(name="mt", bufs=2) as m_pool,
                tc.tile_pool(name="prp", bufs=2) as pr_pool,
                tc.tile_pool(name="pmp", bufs=2) as pm_pool,
                tc.tile_pool(name="dap", bufs=2) as da_pool,
                tc.tile_pool(name="otq", bufs=2) as ot_pool,
                tc.tile_pool(name="odp", bufs=2) as od_pool,
                tc.tile_pool(name="rcp", bufs=1) as rc_pool,
                tc.tile_pool(name="scp", bufs=1, space="PSUM") as sc_pool,
                tc.tile_pool(name="avp", bufs=2, space="PSUM") as av_pool,
                tc.tile_pool(name="dnp", bufs=2, space="PSUM") as dn_pool,
            ):
                wo_sb = wo_pool.tile([P, HL, H], f16, name="wo_sb")
                nc.scalar.dma_start(wo_sb[:], wo[:, :])

                def load_mask(qc):
                    mt = m_pool.tile([P, SB, 512], f16, name="mt", tag="mt")
                    nc.sync.dma_start(mt[:], maskQ_r[:, qc])
                    return mt

                mt = load_mask(0)
                backlog = []

                def oproj_subgroup(oT_src, qc_src, mm, nn):
                    m = 4 * qc_src + mm
                    ops = dn_pool.tile([P, 512], f32, name="ops", tag="dn")
                    for hh in range(HL):
                        nc.tensor.matmul(
                            ops[:],
                            lhsT=oT_src[:, hh, mm * 128:(mm + 1) * 128],
                            rhs=wo_sb[:, hh, nn * 512:(nn + 1) * 512],
                            start=(hh == 0),
                            stop=(hh == HL - 1),
                        )
                    od = od_pool.tile([P, 512], f16, name="od", tag="od")
                    nc.vector.tensor_copy(od[:], ops[:])
                    nc.sync.dma_start(
                        out_r[m][:, nn * 512:(nn + 1) * 512], od[:]
                    )

                for qc in range(NQ):
                    sl = slice(qc * 512, (qc + 1) * 512)
                    oT_qc = ot_pool.tile([P, HL, 512], f16, name="oT", tag="oT")
                    pending = None

                    def flush_pending():
                        dacc_p, ps_av_p, hp, oT_p = pending
                        ps_dn = dn_pool.tile([P, 512], f32, name="dn", tag="dn")
                        for i in range(4):
                            nc.tensor.matmul(
                                ps_dn[:],
                                lhsT=ones_sb[:],
                                rhs=dacc_p[:, i],
                                start=(i == 0),
                                stop=(i == 3),
                            )
                        rc = rc_pool.tile([P, 512], f32, name="rc", tag="rc")
                        nc.vector.reciprocal_approx_fast(rc[:], ps_dn[:])
                        nc.vector.tensor_tensor(
                            oT_p[:, hp], ps_av_p[:], rc[:], MULT
                        )

                    for h in range(HL):
                        ps_av = av_pool.tile([P, 512], f32, name="av", tag="av")
                        dacc = da_pool.tile([P, 4, 512], f16, name="dacc",
                                            tag="dacc")
                        for kq in range(4):
                            pr = pr_pool.tile([P, 4, 512], f16, name="pr",
                                              tag="pr")
                            ps_s = sc_pool.tile([P, 4, 512], f32,
                                                name="ps_s", tag="ps_s")
                            for i in range(4):
                                kb = 4 * kq + i
                                nc.tensor.matmul(
                                    ps_s[:, i],
                                    lhsT=kt_t[h][:, kb * P:(kb + 1) * P],
                                    rhs=qt_t[h][:, sl],
                                    start=True,
                                    stop=True,
                                )
                            nc.scalar.activation(
                                pr[:], ps_s[:], AF.Exp, scale=EXP_SCALE
                            )
                            if kq == 0:
                                nc.vector.tensor_copy(dacc[:], pr[:])
                            else:
                                nc.vector.tensor_tensor(
                                    dacc[:], dacc[:], pr[:], ADD
                                )
                            pm = pm_pool.tile([P, 4, 512], f16, name="pm",
                                              tag="pm")
                            nc.vector.tensor_tensor(
                                pm[:], pr[:], mt[:, 4 * kq:4 * kq + 4, :], MULT
                            )
                            for j in range(4):
                                kb = 4 * kq + j
                                nc.tensor.matmul(
                                    ps_av[:],
                                    lhsT=v_sb[:, kb, h * D:(h + 1) * D],
                                    rhs=pm[:, j],
                                    start=(kq == 0 and j == 0),
                                    stop=(kq == 3 and j == 3),
                                )
                            # delayed denominator for the previous head so the
                            # PE never waits on the DVE accumulation chain
                            if kq == 1 and pending is not None:
                                flush_pending()
                                pending = None
                            if kq == 2 and h == 1 and qc + 1 < NQ:
                                mt_next = load_mask(qc + 1)
                        # two o-proj subgroups of the previous qc per
                        # unit: dependency-free PE work that fills the
                        # exp-wait stalls
                        for _ in range(2):
                            if backlog:
                                oproj_subgroup(*backlog.pop(0))
                        pending = (dacc, ps_av, h, oT_qc)
                    flush_pending()
                    pending = None

                    backlog = [(oT_qc, qc, mm, nn)
                               for mm in range(4) for nn in range(NQ)]
                    if qc == NQ - 1:
                        for sg in backlog:
                            oproj_subgroup(*sg)
                        backlog = []
                    else:
                        mt = mt_next

    nc.compile()
    return nc


def _get_program(with_bv):
    key = ("nc", with_bv)
    if key not in _CACHE:
        _CACHE[key] = _build_program(with_bv)
    return _CACHE[key]


def _host_inputs(x, attention_mask, Wq, bq, Wk, bk, Wv, bv, Wo, bo, with_bv):
    """Build the 8 per-core input maps (core = batch*2 + head_group)."""
    import ml_dtypes

    f8 = ml_dtypes.float8_e4m3
    perm = np.concatenate([np.arange(0, D, 2), np.arange(1, D, 2)])

    inv = (1.0 / (ROPE_BASE ** (np.arange(0, D, 2, dtype=np.float64) / D)))
    t = np.arange(S, dtype=np.float64)
    fr = inv[:, None] * t[None, :]          # (64, S)
    cosP = np.concatenate([np.cos(fr), np.cos(fr)], 0).astype(np.float16)
    # sign folded in: rope = q*cos + swap(q)*sinP with sinP negative on the
    # first 64 partitions (rope[0:64] = q[0:64]c - q[64:128]s)
    sinP = np.concatenate([-np.sin(fr), np.sin(fr)], 0).astype(np.float16)
    ones = np.ones((P, P), np.float16)

    def q8(a):
        return np.clip(a, -240.0, 240.0).astype(f8)

    def w_heads_fp8(W, g):
        # (HL, P, KOP*2*D) fp8, x32, rope-permuted, DoubleRow pair layout
        Wg = (W[:, g * HL * D:(g + 1) * HL * D] * WSCALE).reshape(H, HL, D)
        Wg = Wg[:, :, perm]                                # (H, HL, D)
        Wg = Wg.reshape(KOP, 2, P, HL, D).transpose(3, 2, 0, 1, 4)
        return np.ascontiguousarray(
            q8(Wg.reshape(HL, P, KOP * 2 * D))
        )

    def b_heads_perm(b, g):
        bg = (b[g * HL * D:(g + 1) * HL * D] * WSCALE).reshape(HL, D)
        return np.ascontiguousarray(bg[:, perm].T).astype(np.float32)

    groups = []
    for g in range(G):
        groups.append({
            "wq8": w_heads_fp8(Wq, g),
            "wk8": w_heads_fp8(Wk, g),
            "bqs": b_heads_perm(bq, g),
            "bks": b_heads_perm(bk, g),
            "wv": np.ascontiguousarray(
                Wv[:, g * HL * D:(g + 1) * HL * D].astype(np.float16)
                .reshape(KO, P, 2, 512).transpose(1, 2, 0, 3)
                .reshape(P, 2 * KO * 512)
            ),
            "bv": np.ascontiguousarray(np.broadcast_to(
                bv[g * HL * D:(g + 1) * HL * D], (P, HL * D)
            )).astype(np.float16),
            "wo": np.ascontiguousarray(
                Wo[g * HL * D:(g + 1) * HL * D, :]
                .reshape(HL, D, H).transpose(1, 0, 2).reshape(P, HL * H)
                .astype(np.float16)
            ),
        })

    in_maps = []
    for b in range(B):
        xT = x[b].T                                        # (H, S)
        xtA = np.ascontiguousarray(
            q8(xT.reshape(KOP, 2, P, NQ, 512).transpose(2, 3, 0, 1, 4)
               .reshape(P, KOP * 2 * S))
        )
        xtB = np.ascontiguousarray(
            xT.reshape(KO, P, NQ, 512).transpose(1, 2, 0, 3)
            .reshape(P, KO * S).astype(np.float16)
        )
        maskQ = np.ascontiguousarray(
            attention_mask[b, 0].T.astype(np.float16)
            .reshape(KO, P, NQ, 512).transpose(1, 2, 0, 3)
            .reshape(P, NQ * KO * 512)
        )
        for g in range(G):
            m = dict(groups[g])
            m["xtA"] = xtA
            m["xtB"] = xtB
            m["maskQ"] = maskQ
            m["cosP"] = cosP
            m["sinP"] = sinP
            m["ones"] = ones
            in_maps.append(m)
    return in_maps


def kernel(x, attention_mask, Wq, bq, Wk, bk, Wv, bv, Wo, bo, _trace=False,
           _tmpdir=None):
    from concourse.bass_utils import run_bass_kernel_spmd

    with_bv = bool(np.any(bv))
    nc = _get_program(with_bv)
    in_maps = _host_inputs(
        x, attention_mask, Wq, bq, Wk, bk, Wv, bv, Wo, bo, with_bv
    )
    res = run_bass_kernel_spmd(
        nc, in_maps, list(range(8)), trace=_trace, tmpdir=_tmpdir
    )
    outs = [res.results[c]["out"] for c in range(8)]
    full = np.empty((B, S, H), np.float32)
    for b in range(B):
        full[b] = (outs[2 * b].astype(np.float32)
                   + outs[2 * b + 1].astype(np.float32) + bo[None, :])
    if _trace:
        _CACHE["last_exec_time_ns"] = res.exec_time_ns
        _CACHE["last_results"] = res
    return full
